# revision 12
# baseline (speedup 1.0000x reference)
"""Trainium2 Bass kernel for nn_AccidentDetection (8-core SPMD, batch-parallel).

Math (validated against reference):
  softmax over a size-1 axis is identically 1.0, so al == obj_mask and the
  attention branch (W_ah/W_ac) is dead code. The k-reduction folds through
  the linear layers:
    m[b,n,k]  = sum_d x[b,n,k+1,d]            (== alphas output)
    s1 = sum_k m ; s2 = sum_k m^2
    u  = sum_k m^2 * x[:,:,k+1,:]             (B,N,4096)
    z  = u @ W_obj.T  + b_obj  * s2
    w  = z @ W_obj2.T + b_obj2 * s1           (== attention-weighted obj feat)
    img = x[:,:,0,:] @ W_img.T + b_img
    xl  = [img, w] -> 2-layer LSTM scan over N -> preds
  The LSTM is run layer-by-layer: ih contributions are batched matmuls over
  all N; only the hh matmul is sequential (bf16 weights, gates kept
  transposed so the cell runs on 128 partitions).

Per-core shard: 4 of 32 batch rows. No collectives.
"""
import json
import types
import sys

import numpy as np
import ml_dtypes

B, N, K, D = 32, 100, 20, 4096
DI = 512
H = 512
NCORES = 8
BS = B // NCORES          # batch shard per core
R = BS * N                # frames per core (b-major: r = b*100 + n)
FPB = 6                   # frames per phase-A block
ROWS = FPB * K            # 120 rows
NBLK = (R + FPB - 1) // FPB   # 67 (last block has 4 frames)
# gate-chunk permutation: torch order i,f,g,o -> layout i,f,o,g (128-chunks)
PERM = [0, 1, 2, 3, 4, 5, 6, 7, 12, 13, 14, 15, 8, 9, 10, 11]

_MAXW = 1
_ws_count = [0]


def _split_waits(bir_bytes: bytes) -> bytes:
    """Walrus here accepts only one sync-wait per instruction; hoist extra
    on_wait entries onto preceding same-engine NoOps."""
    d = json.loads(bir_bytes)
    changed = False
    for f in d["functions"]:
        for blk in f["blocks"]:
            out = []
            for i in blk["instructions"]:
                si = i.get("sync_info")
                if si:
                    ow = si.get("on_wait") or []
                    if len(ow) > _MAXW:
                        changed = True
                        extra, keep = ow[:-_MAXW], ow[-_MAXW:]
                        si["on_wait"] = keep
                        for ci in range(0, len(extra), _MAXW):
                            _ws_count[0] += 1
                            out.append({
                                "debug": i.get("debug", 0),
                                "engine": i["engine"],
                                "ins": [], "outs": [],
                                "name": f"WS-{_ws_count[0]}",
                                "opcode": "NoOp",
                                "sync_info": {"on_update": [],
                                              "on_wait": extra[ci:ci + _MAXW]},
                            })
                out.append(i)
            blk["instructions"] = out
    return json.dumps(d).encode() if changed else bir_bytes


def _install_patch():
    import concourse.bass as bass
    if getattr(bass.Bass, "_ws_patched", False):
        return
    orig = bass.Bass.to_json_bytes

    def to_json_bytes(self, *a, **k):
        return _split_waits(orig(self, *a, **k))

    bass.Bass.to_json_bytes = to_json_bytes
    bass.Bass._ws_patched = True


_NC_CACHE = {}
_LAST = None


def _build():
    import concourse.bass as bass
    import concourse.mybir as mybir
    import concourse.tile as tile

    f32 = mybir.dt.float32
    f32r = mybir.dt.float32r
    bf16 = mybir.dt.bfloat16
    AF = mybir.ActivationFunctionType

    def r_(ap):
        return ap.bitcast(f32r)

    nc = bass.Bass()
    # ---- I/O ----
    x = nc.dram_tensor("x", [BS, N, K, D], f32, kind="ExternalInput")
    wimgT = nc.dram_tensor("wimgT", [D, DI], f32, kind="ExternalInput")
    wobjT = nc.dram_tensor("wobjT", [D, DI], f32, kind="ExternalInput")
    wobj2T = nc.dram_tensor("wobj2T", [DI, DI], f32, kind="ExternalInput")
    wih0T = nc.dram_tensor("wih0T", [2 * DI, 4 * H], f32, kind="ExternalInput")
    wih1T = nc.dram_tensor("wih1T", [H, 4 * H], f32, kind="ExternalInput")
    whh0T = nc.dram_tensor("whh0T", [H, 4 * H], bf16, kind="ExternalInput")
    whh1T = nc.dram_tensor("whh1T", [H, 4 * H], bf16, kind="ExternalInput")
    woutT = nc.dram_tensor("woutT", [H, 1], f32, kind="ExternalInput")
    bimg_col = nc.dram_tensor("bimg_col", [128, 4], f32, kind="ExternalInput")
    bsum0_col = nc.dram_tensor("bsum0_col", [128, 16], f32, kind="ExternalInput")
    bsum1_col = nc.dram_tensor("bsum1_col", [128, 16], f32, kind="ExternalInput")
    bobj_row = nc.dram_tensor("bobj_row", [1, DI], f32, kind="ExternalInput")
    bobj2_row = nc.dram_tensor("bobj2_row", [1, DI], f32, kind="ExternalInput")
    bout = nc.dram_tensor("bout", [1, 1], f32, kind="ExternalInput")
    mask01 = nc.dram_tensor("mask01", [ROWS, FPB], f32, kind="ExternalInput")
    ones_row = nc.dram_tensor("ones_row", [1, R], f32, kind="ExternalInput")
    m_out = nc.dram_tensor("m_out", [BS * N * K], f32, kind="ExternalOutput")
    preds_out = nc.dram_tensor("preds_out", [1, R], f32, kind="ExternalOutput")
    u_dramT = nc.dram_tensor("u_dramT", [D, R], f32)

    x2 = x.rearrange("b n k d -> (b n k) d")

    with tile.TileContext(nc) as tc:
        with tc.tile_pool(name="persist", bufs=1) as pp, \
             tc.tile_pool(name="stream", bufs=2) as sp, \
             tc.tile_pool(name="wtiles", bufs=4) as wp:

            # ---- persistent SBUF tiles ----
            mask_t = pp.tile([ROWS, FPB], f32, tag="mask")
            ones_t = pp.tile([1, R], f32r, tag="ones")
            bimg_t = pp.tile([128, 4], f32, tag="bimg")
            bsum0_t = pp.tile([128, 16], f32, tag="bsum0")
            bsum1_t = pp.tile([128, 16], f32, tag="bsum1")
            bobj_t = pp.tile([1, DI], f32, tag="bobj")
            bobj2_t = pp.tile([1, DI], f32, tag="bobj2")
            bout_t = pp.tile([1, 1], f32r, tag="bout")
            s1row = pp.tile([1, R], f32, tag="s1row")
            s2row = pp.tile([1, R], f32, tag="s2row")
            XlT = pp.tile([128, 8 * R], f32, tag="XlT")       # [img(4); w(4)]
            zT = pp.tile([128, 4 * R], f32, tag="zT")
            G0T = pp.tile([128, 16 * R], f32, tag="G0T")
            G1T = pp.tile([128, 16 * R], f32, tag="G1T")
            H0T = pp.tile([128, 4 * R], f32r, tag="H0T")
            H1T = pp.tile([128, 4 * R], f32r, tag="H1T")
            whh0_t = pp.tile([128, 4 * 2048], bf16, tag="whh0")
            whh1_t = pp.tile([128, 4 * 2048], bf16, tag="whh1")
            wout_t = pp.tile([128, 4], f32r, tag="wout")
            cst = pp.tile([128, 16], f32, tag="cst")
            hbf = pp.tile([128, 16], bf16, tag="hbf")

            nc.sync.dma_start(mask_t[:], mask01[:])
            nc.sync.dma_start(ones_t[:], ones_row[:].bitcast(f32r))
            nc.sync.dma_start(bimg_t[:], bimg_col[:])
            nc.sync.dma_start(bsum0_t[:], bsum0_col[:])
            nc.sync.dma_start(bsum1_t[:], bsum1_col[:])
            nc.sync.dma_start(bobj_t[:], bobj_row[:])
            nc.sync.dma_start(bobj2_t[:], bobj2_row[:])
            nc.sync.dma_start(bout_t[:], bout[:].bitcast(f32r))
            nc.sync.dma_start(
                whh0_t[:].rearrange("p (kc g) -> p kc g", kc=4),
                whh0T.rearrange("(kc p) g -> p kc g", p=128))
            nc.sync.dma_start(
                whh1_t[:].rearrange("p (kc g) -> p kc g", kc=4),
                whh1T.rearrange("(kc p) g -> p kc g", p=128))
            nc.sync.dma_start(
                wout_t[:].rearrange("p (kc one) -> p kc one", kc=4),
                woutT.rearrange("(kc p) one -> p kc one", p=128).bitcast(f32r))

            # ================= phase A: stream x, compute m / s / u ========
            with tc.tile_pool(name="psA", bufs=2, space="PSUM") as psA, \
                 tc.tile_pool(name="psS", bufs=1, space="PSUM") as psS:
                psum_s1 = psS.tile([1, R], f32, tag="s1")
                psum_s2 = psS.tile([1, R], f32, tag="s2")
                for j in range(NBLK):
                    nf = min(FPB, R - FPB * j)
                    rows = nf * K
                    xt = sp.tile([ROWS, D], f32, tag="xt")
                    nc.sync.dma_start(
                        xt[:rows, :], x2[ROWS * j: ROWS * j + rows, :])
                    mval = sp.tile([ROWS, 1], f32, tag="mval")
                    mact = sp.tile([ROWS, 1], f32, tag="mact")
                    scr = sp.tile([ROWS, D // 2], bf16, tag="scr")
                    nc.vector.reduce_sum(
                        mval[:rows, :], xt[:rows, 0: D // 2],
                        axis=mybir.AxisListType.X)
                    nc.scalar.activation(
                        scr[:rows, :], xt[:rows, D // 2: D], AF.Copy,
                        accum_out=mact[:rows, :])
                    nc.vector.tensor_tensor(
                        out=mval[:rows, :], in0=mval[:rows, :],
                        in1=mact[:rows, :], op=mybir.AluOpType.add)
                    nc.gpsimd.dma_start(
                        m_out[ROWS * j: ROWS * j + rows],
                        mval[:rows, 0])
                    m2val = sp.tile([ROWS, 1], f32, tag="m2val")
                    nc.vector.tensor_tensor(
                        out=m2val[:rows, :], in0=mval[:rows, :],
                        in1=mval[:rows, :], op=mybir.AluOpType.mult)
                    m2mask = sp.tile([ROWS, FPB], f32, tag="m2mask")
                    nc.vector.tensor_scalar_mul(
                        m2mask[:rows, :nf], mask_t[:rows, :nf],
                        m2val[:rows, 0:1])
                    # s1/s2 via PE: m.T @ mask, (m^2).T @ mask
                    nc.tensor.matmul(
                        psum_s1[:, FPB * j: FPB * j + nf],
                        mval[:rows, :], mask_t[:rows, :nf],
                        start=True, stop=True)
                    nc.tensor.matmul(
                        psum_s2[:, FPB * j: FPB * j + nf],
                        m2val[:rows, :], mask_t[:rows, :nf],
                        start=True, stop=True)
                    # u.T chunks via PE: x_chunk.T @ m2mask -> one PSUM bank
                    pu = psA.tile([128, 32 * FPB], f32, tag="u")
                    for c in range(32):
                        nc.tensor.matmul(
                            pu[:, FPB * c: FPB * c + nf],
                            xt[:rows, 128 * c: 128 * (c + 1)],
                            m2mask[:rows, :nf],
                            start=True, stop=True)
                    stg = sp.tile([128, 32 * FPB], f32, tag="ustg")
                    nc.vector.tensor_copy(stg[:], pu[:])
                    nc.gpsimd.dma_start(
                        u_dramT.rearrange("(c p) r -> p c r", p=128)
                        [:, :, FPB * j: FPB * j + nf],
                        stg[:].rearrange("p (c fr) -> p c fr", fr=FPB)
                        [:, :, :nf])
                nc.vector.tensor_copy(s1row[:], psum_s1[:])
                nc.vector.tensor_copy(s2row[:], psum_s2[:])

            # ================= img & z & w matmuls =========================
            def big_matmul(dst, dst_off, wT_dram, nK, rhs_chunk, bias_col,
                           rank1_lhs, rank1_rhs, tag, wdt=f32r):
                """dst[:, dst_off+400f : +400] = wT.T @ rhs (+rank1) (+bias)"""
                with tc.tile_pool(name=f"ps_{tag}", bufs=1,
                                  space="PSUM") as ps:
                    psum = [ps.tile([128, R], f32, tag=f"{tag}{f}",
                                    name=f"ps_{tag}{f}")
                            for f in range(4)]
                    for c in range(nK):
                        rt = rhs_chunk(c)
                        for f in range(4):
                            wt = wp.tile([128, 128], wdt, tag=f"w_{tag}",
                                         name="wt4")
                            nc.scalar.dma_start(
                                wt[:],
                                wT_dram[128 * c: 128 * (c + 1),
                                        128 * f: 128 * (f + 1)].bitcast(wdt))
                            nc.tensor.matmul(
                                psum[f][:], wt[:], rt,
                                start=(c == 0),
                                stop=(c == nK - 1 and rank1_lhs is None))
                    for f in range(4):
                        if rank1_lhs is not None:
                            nc.tensor.matmul(
                                psum[f][:],
                                rank1_lhs[0:1, 128 * f: 128 * (f + 1)],
                                rank1_rhs[0:1, :],
                                start=False, stop=True)
                        if bias_col is not None:
                            nc.vector.tensor_scalar_add(
                                dst[:, dst_off + R * f: dst_off + R * (f + 1)],
                                psum[f][:], bias_col[:, f: f + 1])
                        else:
                            nc.vector.tensor_copy(
                                dst[:, dst_off + R * f: dst_off + R * (f + 1)],
                                psum[f][:])

            def x0_chunk(c):
                t = sp.tile([128, R], f32r, tag="x0t")
                nc.sync.dma_start(
                    t[:], x[:, :, 0, 128 * c: 128 * (c + 1)]
                    .rearrange("b n d -> d (b n)").bitcast(f32r))
                return t[:]

            def ut_chunk(c):
                t = sp.tile([128, R], f32, tag="ut")
                nc.sync.dma_start(
                    t[:], u_dramT[128 * c: 128 * (c + 1), :])
                return t[:]

            def zt_chunk(c):
                return zT[:, R * c: R * (c + 1)]

            def xlt_chunk(c):
                return XlT[:, R * c: R * (c + 1)]

            def h0t_chunk(c):
                return H0T[:, R * c: R * (c + 1)]

            big_matmul(XlT, 0, wimgT, 32, x0_chunk, bimg_t, None, None, "img")
            big_matmul(zT, 0, wobjT, 32, ut_chunk, None, bobj_t, s2row, "z", wdt=f32)
            big_matmul(XlT, 4 * R, wobj2T, 4, zt_chunk, None, bobj2_t, s1row,
                       "w2", wdt=f32)

            # ================= G0 = Wih0 @ Xl.T + bias =====================
            def big_matmul16(dstT, wT_dram, nK, rhs_chunk, bias_col, tag,
                             wdt=f32r):
                for half in range(2):
                    with tc.tile_pool(name=f"ps_{tag}{half}", bufs=1,
                                      space="PSUM") as ps:
                        psum = [ps.tile([128, R], f32, tag=f"{tag}{f}",
                                        name=f"ps_{tag}{half}{f}")
                                for f in range(8)]
                        for c in range(nK):
                            rt = rhs_chunk(c)
                            for f in range(8):
                                fa = 8 * half + f
                                wt = wp.tile([128, 128], wdt, tag=f"w_{tag}",
                                             name="wt16")
                                nc.scalar.dma_start(
                                    wt[:],
                                    wT_dram[128 * c: 128 * (c + 1),
                                            128 * fa: 128 * (fa + 1)]
                                    .bitcast(wdt))
                                nc.tensor.matmul(
                                    psum[f][:], wt[:], rt,
                                    start=(c == 0), stop=(c == nK - 1))
                        for f in range(8):
                            fa = 8 * half + f
                            nc.vector.tensor_scalar_add(
                                dstT[:, R * fa: R * (fa + 1)],
                                psum[f][:], bias_col[:, fa: fa + 1])

            big_matmul16(G0T, wih0T, 8, xlt_chunk, bsum0_t, "G0", wdt=f32)

            # ================= LSTM scan (shared) ==========================
            def scan(GT, whh_t, HT):
                GTv = GT.rearrange("p (q t) -> p t q", t=N)     # (128,100,64)
                HTv = HT.rearrange("p (q t) -> p t q", t=N)     # (128,100,16)
                nc.gpsimd.memset(cst[:], 0.0)
                nc.gpsimd.memset(hbf[:], 0.0)
                with tc.tile_pool(name="ps_g", bufs=2, space="PSUM") as ps, \
                     tc.tile_pool(name="cellp", bufs=3) as cp:
                    for t in range(N):
                        pg = ps.tile([128, 64], f32, tag="g")
                        for m in range(16):
                            for kc in range(4):
                                nc.tensor.matmul(
                                    pg[:, 4 * m: 4 * (m + 1)],
                                    whh_t[:, 2048 * kc + 128 * m:
                                          2048 * kc + 128 * (m + 1)],
                                    hbf[:, 4 * kc: 4 * (kc + 1)],
                                    start=(kc == 0), stop=(kc == 3))
                        nc.vector.tensor_tensor(
                            out=pg[:], in0=pg[:], in1=GTv[:, t, :],
                            op=mybir.AluOpType.add)
                        sigs = cp.tile([128, 48], f32, tag="sigs")
                        tg = cp.tile([128, 16], f32, tag="tg")
                        tch = cp.tile([128, 16], f32, tag="tch")
                        t1 = cp.tile([128, 16], f32, tag="t1")
                        nc.scalar.activation(sigs[:], pg[:, 0:48], AF.Sigmoid)
                        nc.scalar.activation(tg[:], pg[:, 48:64], AF.Tanh)
                        nc.vector.tensor_tensor(
                            out=t1[:], in0=sigs[:, 0:16], in1=tg[:],
                            op=mybir.AluOpType.mult)
                        nc.vector.tensor_tensor(
                            out=cst[:], in0=sigs[:, 16:32], in1=cst[:],
                            op=mybir.AluOpType.mult)
                        nc.vector.tensor_tensor(
                            out=cst[:], in0=cst[:], in1=t1[:],
                            op=mybir.AluOpType.add)
                        nc.scalar.activation(tch[:], cst[:], AF.Tanh)
                        nc.vector.tensor_tensor(
                            out=HTv[:, t, :], in0=sigs[:, 32:48], in1=tch[:],
                            op=mybir.AluOpType.mult)
                        nc.vector.tensor_copy(hbf[:], HTv[:, t, :])

            scan(G0T, whh0_t, H0T)
            big_matmul16(G1T, wih1T, 4, h0t_chunk, bsum1_t, "G1")
            scan(G1T, whh1_t, H1T)

            # ================= preds =======================================
            with tc.tile_pool(name="ps_p", bufs=1, space="PSUM") as ps:
                pp_ = ps.tile([1, R], f32, tag="p")
                for kc in range(4):
                    nc.tensor.matmul(
                        pp_[:], wout_t[:, kc: kc + 1],
                        H1T[:, R * kc: R * (kc + 1)],
                        start=(kc == 0), stop=False)
                nc.tensor.matmul(
                    pp_[:], bout_t[:], ones_t[:],
                    start=False, stop=True)
                pp_sb = sp.tile([1, R], f32, tag="psb")
                nc.vector.tensor_copy(pp_sb[:], pp_[:])
                nc.gpsimd.dma_start(preds_out[:], pp_sb[:])

    return nc


def _prep_consts(inputs):
    f32 = np.float32
    bf16 = ml_dtypes.bfloat16

    def perm_rows(w):
        return w.reshape(16, 128, -1)[PERM].reshape(4 * H, -1)

    def perm_vec(v):
        return v.reshape(16, 128)[PERM].reshape(-1)

    wih0 = perm_rows(inputs["W_ih0"])
    wih1 = perm_rows(inputs["W_ih1"])
    whh0 = perm_rows(inputs["W_hh0"])
    whh1 = perm_rows(inputs["W_hh1"])
    bsum0 = perm_vec(inputs["b_ih0"] + inputs["b_hh0"])
    bsum1 = perm_vec(inputs["b_ih1"] + inputs["b_hh1"])
    mask = np.zeros((ROWS, FPB), f32)
    for j in range(FPB):
        mask[j * K + 1:(j + 1) * K, j] = 1.0
    c = {
        "wimgT": np.ascontiguousarray(inputs["W_img"].T.astype(f32)),
        "wobjT": np.ascontiguousarray(inputs["W_obj"].T.astype(f32)),
        "wobj2T": np.ascontiguousarray(inputs["W_obj2"].T.astype(f32)),
        "wih0T": np.ascontiguousarray(wih0.T.astype(f32)),
        "wih1T": np.ascontiguousarray(wih1.T.astype(f32)),
        "whh0T": np.ascontiguousarray(whh0.T.astype(f32)).astype(bf16),
        "whh1T": np.ascontiguousarray(whh1.T.astype(f32)).astype(bf16),
        "woutT": np.ascontiguousarray(inputs["W_out"].T.astype(f32)),
        "bimg_col": np.ascontiguousarray(
            inputs["b_img"].reshape(4, 128).T.astype(f32)),
        "bsum0_col": np.ascontiguousarray(bsum0.reshape(16, 128).T.astype(f32)),
        "bsum1_col": np.ascontiguousarray(bsum1.reshape(16, 128).T.astype(f32)),
        "bobj_row": inputs["b_obj"].reshape(1, DI).astype(f32),
        "bobj2_row": inputs["b_obj2"].reshape(1, DI).astype(f32),
        "bout": inputs["b_out"].reshape(1, 1).astype(f32),
        "mask01": mask,
        "ones_row": np.ones((1, R), f32),
    }
    return c


def kernel(**inputs):
    _install_patch()
    from concourse.bass_utils import run_bass_kernel_spmd

    if "nc" not in _NC_CACHE:
        _NC_CACHE["nc"] = _build()
    nc = _NC_CACHE["nc"]

    consts = _prep_consts({k: np.asarray(v) for k, v in inputs.items()})
    x = np.ascontiguousarray(np.asarray(inputs["x"], dtype=np.float32))
    in_maps = []
    for i in range(NCORES):
        m = dict(consts)
        m["x"] = np.ascontiguousarray(x[BS * i: BS * (i + 1)])
        in_maps.append(m)

    bres = run_bass_kernel_spmd(nc, in_maps, list(range(NCORES)))
    global _LAST
    _LAST = bres
    res = bres.results

    preds = np.concatenate(
        [res[i]["preds_out"].reshape(BS, N) for i in range(NCORES)], axis=0)
    m_full = np.concatenate(
        [res[i]["m_out"].reshape(BS, N, K)[:, :, 1:] for i in range(NCORES)],
        axis=0)                                   # (32,100,19)
    alphas = np.ascontiguousarray(
        m_full.transpose(1, 0, 2)[..., None])     # (100,32,19,1)
    return alphas, preds
